# revision 18
# baseline (speedup 1.0000x reference)
"""Distributed Trainium2 kernel for AdaptiveLinearWithChannel (MoE-routed
batched matmul):  out[t] = x[t] @ weight[indices[t]] + bias

Strategy (expert-parallel per the sharding hint): the tile dimension is
sharded 64-tiles-per-core across 8 NeuronCores.  The indices gather is
resolved during host-side sharding — each core receives its 64 x-tiles
plus its 64 *pre-gathered* weight tiles, so routing is device-local and
no collectives are needed.  Inputs are converted to bf16 on the host
(halves DMA traffic, 4x TensorE throughput; fp32 PSUM accumulation keeps
rel-err ~3e-3).  Each tile computes out^T[o,p] = w[i,o]^T @ x^T[i,p] as
2 accumulating matmuls (K=256 split in 2) x 2 output chunks (D_out=256
split in 2), N=512 points at the fp32-PSUM bank limit.  The host
pre-transposes x so the contraction dim lands on SBUF partitions.

Raw-bass pipeline (TileContext's multi-wait drain trips this walrus
build), all 5 engines:
  SP     : HWDGE ring, x in-DMAs
  GpSimd : SWDGE ring, w in-DMAs + bias preamble (parallel with x)
  PE     : 2x2 matmuls per tile into rotating PSUM banks (8 in flight)
  DVE    : PSUM -> SBUF bf16 copy + per-partition bias add (even groups)
  ACT    : same for odd groups, plus HWDGE out-DMAs

Measured on 8 axon-tunneled trn2 cores: ~116-133 us max-of-8-cores
(~119-122 us per-core mean) vs a ~118 us floor: 13 us fixed NEFF
preamble + 40 MiB/core of HBM traffic at the ~358-380 GB/s per-core
limit, with DMA 93-95% occupied.  l2 rel err vs the f32 reference is
2.9e-3 (gate 2e-2).  A cheap full-coverage column-sum integrity check
retries the rare transient device corruption.
"""

import numpy as np
import ml_dtypes

import concourse.bass as bass
import concourse.mybir as mybir
from concourse.bass_utils import run_bass_kernel_spmd

BF16 = ml_dtypes.bfloat16

N_CORES = 8
NUM_TILES = 512
N_POINTS = 512          # free dim N of each matmul
D_IN = 256              # contraction, 2 chunks of 128
D_OUT = 256             # output partitions, 2 chunks of 128
CHANNELS = 1024
TPC = NUM_TILES // N_CORES   # 64 tiles per core
SUP = 2                      # tiles per super-tile (DMA batch)
NSUP = TPC // SUP            # super-tiles per core
NBUF = 6                     # SBUF buffer sets (pipeline depth)
GROUPS_PER_SUP = SUP * 2     # psum groups per super-tile
OUT_BF16 = True              # store out as bf16 (host upconverts); halves out traffic
W_ON_GPSIMD = True           # issue w-DMAs from the SWDGE ring (parallel with x)

_cache = {}


def _build_nc():
    bf = mybir.dt.bfloat16
    f32 = mybir.dt.float32
    nc = bass.Bass()

    # x_dev[s, p, c, t2, f]   = x[4s+t2, f, 128c+p]   (pre-transposed, bf16)
    # w_dev[s, p, c, t2, o]   = weight[idx[4s+t2], 128c+p, o]
    # out_dev[s, po, t2, j, f] = out[4s+t2, f, 128j+po]  (t2-outermost so
    #   per-t2 out-DMAs stay fully contiguous per partition)
    x_ext = nc.declare_dram_parameter("x", [NSUP, 128, 2, SUP, N_POINTS], bf, isOutput=False)
    w_ext = nc.declare_dram_parameter("w", [NSUP, 128, 2, SUP, D_OUT], bf, isOutput=False)
    b_ext = nc.declare_dram_parameter("b", [128, 2], f32, isOutput=False)
    out_ext = nc.declare_dram_parameter("out", [NSUP, 128, SUP, 2, N_POINTS],
                                        bf if OUT_BF16 else f32, isOutput=True)

    import contextlib
    ctx = contextlib.ExitStack()
    x_sb = [ctx.enter_context(nc.sbuf_tensor(f"x_sb{i}", [128, 2, SUP, N_POINTS], bf)) for i in range(NBUF)]
    w_sb = [ctx.enter_context(nc.sbuf_tensor(f"w_sb{i}", [128, 2, SUP, D_OUT], bf)) for i in range(NBUF)]
    o_sb = [ctx.enter_context(nc.sbuf_tensor(f"o_sb{i}", [128, SUP, 2, N_POINTS],
                                             bf if OUT_BF16 else f32)) for i in range(NBUF)]
    bias_sb = ctx.enter_context(nc.sbuf_tensor("bias_sb", [128, 2], f32))
    psum = [ctx.enter_context(nc.psum_tensor(f"ps{i}", [128, N_POINTS], f32)) for i in range(8)]

    # DMA semaphores are per buffer-slot: a then_inc(sem, 16) lands as 16
    # independent +1s from the SDMA engines, so cumulative intermediate
    # thresholds on a sem with 2+ transfers in flight can fire on a mix of
    # them.  Per-slot sems are only ever waited at their full total, and
    # slot reuse quiesces the previous use's incs first.  Compute sems
    # (sem_pe/sem_dve: single in-order producer) are safe cumulative.
    with ctx:
        with (
            contextlib.ExitStack() as sems,
            nc.Block() as block,
        ):
            sem_x = [sems.enter_context(nc.semaphore(f"sem_x{b}")) for b in range(NBUF)]
            sem_w = [sems.enter_context(nc.semaphore(f"sem_w{b}")) for b in range(NBUF)]
            sem_o = [sems.enter_context(nc.semaphore(f"sem_o{b}")) for b in range(NBUF)]
            sem_bias = sems.enter_context(nc.semaphore("sem_bias"))
            sem_pe = sems.enter_context(nc.semaphore("sem_pe"))
            sem_dve = sems.enter_context(nc.semaphore("sem_dve"))
            sem_acp = sems.enter_context(nc.semaphore("sem_acp"))

            NG = NSUP * GROUPS_PER_SUP
            # copies alternate DVE (j=0) / ACT (j=1); bank sg%8 reuse needs
            # the consumer of group sg-8 done -> (sem, count) per group
            copy_done = []
            n_dve = n_acp = 0
            for sg in range(NG):
                if sg % 2 == 0:
                    n_dve += 1
                    copy_done.append((sem_dve, n_dve))
                else:
                    n_acp += 1
                    copy_done.append((sem_acp, n_acp))

            @block.sync
            def _(sp):
                for s in range(NSUP):
                    if s >= NBUF:
                        # buffer s%NBUF free once PE finished super-tile s-NBUF
                        sp.wait_ge(sem_pe, GROUPS_PER_SUP * (s - NBUF + 1))
                    b = s % NBUF
                    sp.dma_start(x_sb[b][:], x_ext[s]).then_inc(sem_x[b], 16)
                    if not W_ON_GPSIMD:
                        sp.dma_start(w_sb[b][:], w_ext[s]).then_inc(sem_x[b], 16)
                        if s == 0:
                            sp.dma_start(bias_sb[:], b_ext[:]).then_inc(sem_bias, 16)

            if W_ON_GPSIMD:
                @block.gpsimd
                def _(gp):
                    gp.dma_start(bias_sb[:], b_ext[:]).then_inc(sem_bias, 16)
                    for s in range(NSUP):
                        if s >= NBUF:
                            gp.wait_ge(sem_pe, GROUPS_PER_SUP * (s - NBUF + 1))
                        b = s % NBUF
                        gp.dma_start(w_sb[b][:], w_ext[s]).then_inc(sem_w[b], 16)

            @block.tensor
            def _(pe):
                for s in range(NSUP):
                    b, u = s % NBUF, s // NBUF
                    if W_ON_GPSIMD:
                        pe.wait_ge(sem_x[b], 16 * (u + 1))
                        pe.wait_ge(sem_w[b], 16 * (u + 1))
                    else:
                        pe.wait_ge(sem_x[b], 32 * (u + 1))
                    for t2 in range(SUP):
                        for j in range(2):
                            sg = s * GROUPS_PER_SUP + t2 * 2 + j
                            if sg >= 8:
                                csem, cnt = copy_done[sg - 8]
                                pe.wait_ge(csem, cnt)
                            ps = psum[sg % 8]
                            pe.matmul(ps[:], w_sb[b][:, 0, t2, j * 128:(j + 1) * 128],
                                      x_sb[b][:, 0, t2, :], start=True, stop=False)
                            pe.matmul(ps[:], w_sb[b][:, 1, t2, j * 128:(j + 1) * 128],
                                      x_sb[b][:, 1, t2, :], start=False, stop=True
                                      ).then_inc(sem_pe, 1)

            @block.vector
            def _(dve):
                dve.wait_ge(sem_bias, 16)
                for s in range(NSUP):
                    b, u = s % NBUF, s // NBUF
                    if s >= NBUF:
                        # o_sb buffer free once its previous out-DMA completed
                        dve.wait_ge(sem_o[b], 16 * u)
                    for t2 in range(SUP):
                        sg = s * GROUPS_PER_SUP + t2 * 2  # j = 0
                        dve.wait_ge(sem_pe, sg + 1)
                        dve.tensor_scalar_add(o_sb[b][:, t2, 0, :], psum[sg % 8][:],
                                              bias_sb[:, 0:1]).then_inc(sem_dve, 1)

            @block.scalar
            def _(act):
                act.wait_ge(sem_bias, 16)
                for s in range(NSUP):
                    b, u = s % NBUF, s // NBUF
                    if s >= NBUF:
                        act.wait_ge(sem_o[b], 16 * u)
                    for t2 in range(SUP):
                        sg = s * GROUPS_PER_SUP + t2 * 2 + 1  # j = 1
                        act.wait_ge(sem_pe, sg + 1)
                        act.activation(o_sb[b][:, t2, 1, :], psum[sg % 8][:],
                                       mybir.ActivationFunctionType.Identity,
                                       bias=bias_sb[:, 1:2]).then_inc(sem_acp, 1)
                    # one out-DMA per super-tile: finer-grained (per-t2)
                    # out-DMAs measured ~9% SLOWER — per-transfer ring
                    # overhead on the ACT HWDGE ring dominates the shorter
                    # tail.  Explicit sems even for ACT's own copies —
                    # dma_start only rings the DGE doorbell, its SBUF
                    # reads race the ACT pipeline otherwise.
                    act.wait_ge(sem_acp, SUP * (s + 1))
                    act.wait_ge(sem_dve, SUP * (s + 1))
                    act.dma_start(out_ext[s], o_sb[b][:]).then_inc(sem_o[b], 16)

    return nc


def _pack_core(x_core_f32, w_gathered_bf16, bias_f32):
    """Host-side repack of one core's shard into the device layouts."""
    xb = x_core_f32.astype(BF16)                       # [64, 512, 256]
    x_dev = np.ascontiguousarray(
        xb.reshape(NSUP, SUP, N_POINTS, 2, 128).transpose(0, 4, 3, 1, 2))
    w_dev = np.ascontiguousarray(
        w_gathered_bf16.reshape(NSUP, SUP, 2, 128, D_OUT).transpose(0, 3, 2, 1, 4))
    b_dev = np.ascontiguousarray(bias_f32.reshape(2, 128).T.astype(np.float32))
    return x_dev, w_dev, b_dev


def _unpack_core(out_dev):
    # [s, po, t2, j, f] -> [s, t2, f, j, po] -> [64, 512, 256]
    return out_dev.transpose(0, 2, 4, 3, 1).reshape(TPC, N_POINTS, D_OUT).astype(np.float32)


def kernel(x, indices, weight, bias):
    x = np.asarray(x, dtype=np.float32)
    indices = np.asarray(indices).astype(np.int64)
    weight = np.asarray(weight, dtype=np.float32)
    bias = np.asarray(bias, dtype=np.float32)

    if "nc" not in _cache:
        _cache["nc"] = _build_nc()
    nc = _cache["nc"]

    weight_bf = weight.astype(BF16)
    in_maps = []
    for k in range(N_CORES):
        sl = slice(k * TPC, (k + 1) * TPC)
        wg = weight_bf[indices[sl]]                    # device-local gather, on host
        x_dev, w_dev, b_dev = _pack_core(x[sl], wg, bias)
        in_maps.append({"x": x_dev, "w": w_dev, "b": b_dev})

    # Integrity reference: column-sums are linear in the points axis, so
    # out[t].sum(axis=0) == (sum_p x_bf[t]) @ w_bf[idx[t]] + P*bias per
    # tile (up to bf16 rounding noise ~3e-3).  Full tile coverage at ~1%
    # of the compute — catches the rare transient device corruption.
    sx = x.astype(BF16).astype(np.float32).sum(axis=1)          # [T, D_in]
    w_gath = weight_bf[indices].astype(np.float32)              # [T, D_in, D_out]
    colsum_ref = np.einsum("ti,tio->to", sx, w_gath) + N_POINTS * bias[0, 0]
    ref_norm = np.linalg.norm(colsum_ref, axis=1) + 1e-6

    # retry: the remote device occasionally hits a transient failure —
    # either an NRT error (exception) or, rarely, corrupted output blocks
    last_err = None
    out = None
    for attempt in range(4):
        try:
            res = run_bass_kernel_spmd(nc, in_maps, core_ids=list(range(N_CORES)))
        except Exception as e:  # noqa: BLE001
            last_err = e
            import time
            time.sleep(5.0 * (attempt + 1))
            continue
        cand = np.empty((NUM_TILES, N_POINTS, D_OUT), dtype=np.float32)
        for k in range(N_CORES):
            cand[k * TPC:(k + 1) * TPC] = _unpack_core(res.results[k]["out"])
        per_tile_rel = np.linalg.norm(cand.sum(axis=1) - colsum_ref, axis=1) / ref_norm
        if per_tile_rel.max() < 3e-2:
            out = cand
            break
        last_err = RuntimeError(
            f"integrity check failed: max per-tile colsum rel err "
            f"{per_tile_rel.max():.3e} on tiles {np.where(per_tile_rel >= 3e-2)[0][:8]}")
    if out is None:
        raise last_err
    return out
